# revision 36
# baseline (speedup 1.0000x reference)
"""MoE gate kernel for TRN2: logits = h @ W.T + bias; softmax; top-2; renorm.

Data-parallel over 8 NeuronCores: token dim B=16384 sharded to 2048/core,
weight (64, 4096) + bias replicated.

Near-exact fp32 matmul via fp16 splitting: h = h0 + h1 with h0 = fp16(h)
and h1 = bf16(h - h0); W = w0 + w1 with w0 = fp16(W) and w1 scaled by
2^11 into fp16 range (w1s = fp16((W - w0) * 2048)).
  pass A (fp16): [w0 | w1s]^T @ h0 -> w0.h0 (rows 0:64) and
                 2048 * w1.h0 (rows 64:128, descaled in the combine)
  pass B (bf16): bf16(W)^T @ bf16(h1)
All products are exact in the PE (fp32 PSUM accumulation); only the bf16
storage of h1 rounds, reproducing fp32 logits to ~1e-6 so the top-2
indices match the fp32 reference, while both passes stream 2-byte data
at the full 1 cycle/row PE rate (4x the fp32 matmul rate).

Pipeline per core: h loaded naturally in half-depth tiles [128, 2048]
(frees buffers mid-chunk for deep DMA prefetch; next chunk's loads are
emitted before this chunk's output stores so the sync queue never
stalls). PE fp32-transposes build hT blocks; scalar rounds PSUM->SBUF
to f32r (h0), vector computes the bf16 residual (h1); matmuls run PIPE
d-blocks behind the transposes. W setup batches all 32 transposes into
4 borrowed PSUM tiles then splits with 4 wide copies, overlapped with
the first h DMAs. Top-2 via vector max8/idx8; renorm w1=1/(1+e),
w2=e/(1+e) with e=exp(l2-l1) -- softmax-renorm restricted to the top 2.
"""
import numpy as np
import concourse.bacc as bacc
import concourse.mybir as mybir
from concourse.tile import TileContext
from concourse.bass_utils import run_bass_kernel_spmd
from concourse.masks import make_identity

N_CORES = 8
B = 16384
D = 4096
E = 64
B_SHARD = B // N_CORES      # 2048
CHUNK = 512
N_CHUNKS = B_SHARD // CHUNK  # 4
DBLK = D // 128              # 32
TSUB = CHUNK // 128          # 4
HK = 4                       # h tiles split in 4 along depth (d)
DHALF = DBLK // HK           # 8 d-blocks per quarter
PIPE = 2                     # transpose->matmul software pipeline offset

F32 = mybir.dt.float32
F32R = mybir.dt.float32r
F16 = mybir.dt.float16
BF16 = mybir.dt.bfloat16
U32 = mybir.dt.uint32
I32 = mybir.dt.int32
AF = mybir.ActivationFunctionType


def _build():
    nc = bacc.Bacc("TRN2", target_bir_lowering=False, debug=False,
                   num_devices=N_CORES)
    h_d = nc.dram_tensor("h", [B_SHARD, D], F32, kind="ExternalInput")
    w_d = nc.dram_tensor("weight", [E, D], F32, kind="ExternalInput")
    b_d = nc.dram_tensor("bias", [E], F32, kind="ExternalInput")
    ow_d = nc.dram_tensor("topk_w", [B_SHARD, 2], F32, kind="ExternalOutput")
    oi_d = nc.dram_tensor("topk_idx", [B_SHARD, 2], I32, kind="ExternalOutput")

    with TileContext(nc) as tc:
        with (
            tc.tile_pool(name="const", bufs=1) as constp,
            tc.tile_pool(name="hnat", bufs=2) as hnatp,
            tc.tile_pool(name="ht0", bufs=4) as ht0p,
            tc.tile_pool(name="ht1", bufs=4) as ht1p,
            tc.tile_pool(name="small", bufs=2) as smallp,
            tc.tile_pool(name="outp", bufs=2) as outp,
        ):
            hn_live = {}

            def emit_h_dmas(c):
                for k in range(HK):
                    for s in range(TSUB):
                        t = hnatp.tile([128, D // HK], F32,
                                       name=f"hn_{c}_{s}_{k}",
                                       tag=f"hn_{s}_{k}")
                        t0 = c * CHUNK + s * 128
                        d0 = k * (D // HK)
                        nc.sync.dma_start(out=t[:],
                                          in_=h_d[t0:t0 + 128,
                                                  d0:d0 + D // HK])
                        hn_live[(c, s, k)] = t

            bias_sb = constp.tile([E, 1], F32, name="bias_sb")
            nc.sync.dma_start(out=bias_sb[:],
                              in_=b_d.ap().rearrange("(e o) -> e o", o=1))
            wnat = constp.tile([E, D], F32, name="wnat")
            for q in range(4):
                nc.sync.dma_start(out=wnat[:, 1024 * q:1024 * (q + 1)],
                                  in_=w_d[:, 1024 * q:1024 * (q + 1)])
            emit_h_dmas(0)

            ident = constp.tile([128, 128], F32, name="ident")
            make_identity(nc, ident[:])

            # --- W setup: all 32 W^T blocks into 4 borrowed PSUM tiles,
            # then 4 wide copies per output (f32r hi, f32r lo, bf16) ---
            wcat = constp.tile([128, DBLK * 128], F16, name="wcat")
            wb = constp.tile([128, DBLK * E], BF16, name="wb")
            wtmp = constp.tile([128, CHUNK], F32, name="wtmp")
            with tc.tile_pool(name="wps", bufs=4, space="PSUM") as wpsp:
                wps = [wpsp.tile([128, CHUNK], F32, name=f"wps_{j}", tag="wps")
                       for j in range(4)]
                wcv = wcat[:].rearrange("p (d t) -> p d t", t=128)
                for d in range(DBLK):
                    j, m = d // 8, d % 8
                    nc.tensor.transpose(wps[j][:, E * m:E * (m + 1)],
                                        wnat[:, 128 * d:128 * (d + 1)],
                                        ident[0:E, 0:E])
                    if m == 7:
                        c0 = wcv[:, 8 * j:8 * (j + 1), 0:E]
                        c1 = wcv[:, 8 * j:8 * (j + 1), E:128]
                        nc.scalar.copy(c0, wps[j][:])
                        nc.vector.tensor_sub(wtmp[:], wps[j][:], c0)
                        nc.vector.tensor_scalar(c1, wtmp[:], 2048.0,
                                                scalar2=None,
                                                op0=mybir.AluOpType.mult)
                        nc.scalar.copy(wb[:, CHUNK * j:CHUNK * (j + 1)],
                                       wps[j][:])

            with (
                tc.tile_pool(name="lpA", bufs=2, space="PSUM") as lpAp,
                tc.tile_pool(name="lpB", bufs=2, space="PSUM") as lpBp,
                tc.tile_pool(name="ltps", bufs=2, space="PSUM") as ltpsp,
                tc.tile_pool(name="tps", bufs=2, space="PSUM") as tpsp,
            ):
                lp_live = {}

                def emit_tail(c):
                    # logits^T = (w1.h0 + bias) + w0.h0 + bf16(w).h1
                    lpA, lpB = lp_live.pop(c)
                    t1 = smallp.tile([E, CHUNK], F32, name=f"t1_{c}",
                                     tag="t1")
                    t2 = smallp.tile([E, CHUNK], F32, name=f"t2_{c}",
                                     tag="tmp")
                    lsb = smallp.tile([E, CHUNK], F32, name=f"lsb_{c}",
                                      tag="tmp")
                    nc.scalar.activation(t1[:], lpA[0:E, :], AF.Identity,
                                         bias=bias_sb[:])
                    nc.vector.scalar_tensor_tensor(
                        t2[:], lpA[E:128, :], 1.0 / 2048.0, t1[:],
                        op0=mybir.AluOpType.mult, op1=mybir.AluOpType.add)
                    nc.vector.tensor_add(lsb[:], t2[:], lpB[:])

                    # back to [128 tok, 64 e]; top-2; renorm (chunk-wide)
                    m8 = smallp.tile([128, 8 * TSUB], F32, name=f"m8_{c}",
                                     tag="m8")
                    i8 = smallp.tile([128, 8 * TSUB], U32, name=f"i8_{c}",
                                     tag="i8")
                    lt = ltpsp.tile([128, TSUB * E], F32, name=f"lt_{c}",
                                    tag="lt")
                    for s in range(TSUB):
                        nc.tensor.transpose(lt[:, E * s:E * (s + 1)],
                                            lsb[:, 128 * s:128 * (s + 1)],
                                            ident[0:E, 0:E])
                    for s in range(TSUB):
                        nc.vector.max_with_indices(m8[:, 8 * s:8 * s + 8],
                                                   i8[:, 8 * s:8 * s + 8],
                                                   lt[:, E * s:E * (s + 1)])

                    oww = outp.tile([128, 2 * TSUB], F32, name=f"oww_{c}",
                                    tag="oww")
                    oii = outp.tile([128, 2 * TSUB], I32, name=f"oii_{c}",
                                    tag="oii")
                    dd = smallp.tile([128, TSUB], F32, name=f"dd_{c}",
                                     tag="dd")
                    e2 = smallp.tile([128, TSUB], F32, name=f"e2_{c}",
                                     tag="e2")
                    den = smallp.tile([128, TSUB], F32, name=f"den_{c}",
                                      tag="den")
                    m8v = m8[:].rearrange("p (s e) -> p s e", e=8)
                    i8v = i8[:].rearrange("p (s e) -> p s e", e=8)
                    owv = oww[:].rearrange("p (s c) -> p s c", c=2)
                    nc.vector.tensor_sub(dd[:], m8v[:, :, 1:2],
                                         m8v[:, :, 0:1])
                    nc.scalar.activation(e2[:], dd[:], AF.Exp)
                    nc.vector.tensor_scalar(den[:], e2[:], 1.0, scalar2=None,
                                            op0=mybir.AluOpType.add)
                    nc.vector.reciprocal(owv[:, :, 0:1], den[:])
                    nc.vector.tensor_mul(owv[:, :, 1:2], e2[:],
                                         owv[:, :, 0:1])
                    nc.vector.tensor_copy(oii[:], i8v[:, :, 0:2].bitcast(I32))

                    t0 = c * CHUNK
                    nc.sync.dma_start(
                        out=ow_d[t0:t0 + CHUNK, :].rearrange(
                            "(s p) c -> p s c", p=128),
                        in_=oww[:])
                    nc.sync.dma_start(
                        out=oi_d[t0:t0 + CHUNK, :].rearrange(
                            "(s p) c -> p s c", p=128),
                        in_=oii[:])

                for c in range(N_CHUNKS):
                    if c + 1 < N_CHUNKS:
                        emit_h_dmas(c + 1)
                    hn = [[hn_live.pop((c, s, k)) for k in range(HK)]
                          for s in range(TSUB)]

                    lpA = lpAp.tile([128, CHUNK], F32, name=f"lpA_{c}",
                                    tag="lpA")
                    lpB = lpBp.tile([E, CHUNK], F32, name=f"lpB_{c}",
                                    tag="lpB")
                    lp_live[c] = (lpA, lpB)
                    ht_live = {}
                    for step in range(DBLK + PIPE):
                        if step < DBLK:
                            d = step
                            k, dk = d // DHALF, d % DHALF
                            tp = tpsp.tile([128, CHUNK], F32,
                                           name=f"tp_{c}_{d}", tag="tp")
                            for s in range(TSUB):
                                nc.tensor.transpose(
                                    tp[:, 128 * s:128 * (s + 1)],
                                    hn[s][k][:, 128 * dk:128 * (dk + 1)],
                                    ident[:])
                            ht0 = ht0p.tile([128, CHUNK], F16,
                                            name=f"ht0_{c}_{d}", tag="ht0")
                            ht1 = ht1p.tile([128, CHUNK], BF16,
                                            name=f"ht1_{c}_{d}", tag="ht1")
                            nc.scalar.copy(ht0[:], tp[:])
                            nc.vector.tensor_sub(ht1[:], tp[:], ht0[:])
                            ht_live[d] = (ht0, ht1)
                        if step >= PIPE:
                            d = step - PIPE
                            ht0, ht1 = ht_live.pop(d)
                            nc.tensor.matmul(lpA[:],
                                             wcat[:, 128 * d:128 * (d + 1)],
                                             ht0[:], start=(d == 0),
                                             stop=(d == DBLK - 1))
                            nc.tensor.matmul(lpB[:],
                                             wb[:, E * d:E * (d + 1)],
                                             ht1[:], start=(d == 0),
                                             stop=(d == DBLK - 1))
                        if step == 3 and c > 0:
                            emit_tail(c - 1)
                emit_tail(N_CHUNKS - 1)

    nc.compile()
    return nc


_NC = None


def _get_nc():
    global _NC
    if _NC is None:
        _NC = _build()
    return _NC


def run(h, weight, bias, trace=False):
    nc = _get_nc()
    h = np.ascontiguousarray(h, dtype=np.float32)
    weight = np.ascontiguousarray(weight, dtype=np.float32)
    bias = np.ascontiguousarray(bias, dtype=np.float32)
    in_maps = [{"h": h[i * B_SHARD:(i + 1) * B_SHARD], "weight": weight,
                "bias": bias} for i in range(N_CORES)]
    res = run_bass_kernel_spmd(nc, in_maps, list(range(N_CORES)), trace=trace)
    tw = np.concatenate([res.results[i]["topk_w"] for i in range(N_CORES)], 0)
    ti = np.concatenate([res.results[i]["topk_idx"] for i in range(N_CORES)], 0)
    return (tw.astype(np.float32), ti.astype(np.int32)), res


def kernel(h, weight, bias):
    (tw, ti), _ = run(h, weight, bias)
    return tw, ti


# revision 37
# speedup vs baseline: 1.0082x; 1.0082x over previous
"""MoE gate kernel for TRN2: logits = h @ W.T + bias; softmax; top-2; renorm.

Data-parallel over 8 NeuronCores: token dim B=16384 sharded to 2048/core,
weight (64, 4096) + bias replicated.

Near-exact fp32 matmul via fp16 splitting: h = h0 + h1 with h0 = fp16(h)
and h1 = bf16(h - h0); W = w0 + w1 with w0 = fp16(W) and w1 scaled by
2^11 into fp16 range (w1s = fp16((W - w0) * 2048)).
  pass A (fp16): [w0 | w1s]^T @ h0 -> w0.h0 (rows 0:64) and
                 2048 * w1.h0 (rows 64:128, descaled in the combine)
  pass B (bf16): bf16(W)^T @ bf16(h1)
All products are exact in the PE (fp32 PSUM accumulation); only the bf16
storage of h1 rounds, reproducing fp32 logits to ~1e-6 so the top-2
indices match the fp32 reference, while both passes stream 2-byte data
at the full 1 cycle/row PE rate (4x the fp32 matmul rate).

Pipeline per core: h loaded naturally in half-depth tiles [128, 2048]
(frees buffers mid-chunk for deep DMA prefetch; next chunk's loads are
emitted before this chunk's output stores so the sync queue never
stalls). PE fp32-transposes build hT blocks; scalar rounds PSUM->SBUF
to f32r (h0), vector computes the bf16 residual (h1); matmuls run PIPE
d-blocks behind the transposes. W setup batches all 32 transposes into
4 borrowed PSUM tiles then splits with 4 wide copies, overlapped with
the first h DMAs. Top-2 via vector max8/idx8; renorm w1=1/(1+e),
w2=e/(1+e) with e=exp(l2-l1) -- softmax-renorm restricted to the top 2.
"""
import numpy as np
import concourse.bacc as bacc
import concourse.mybir as mybir
from concourse.tile import TileContext
from concourse.bass_utils import run_bass_kernel_spmd
from concourse.masks import make_identity

N_CORES = 8
B = 16384
D = 4096
E = 64
B_SHARD = B // N_CORES      # 2048
CHUNK = 512
N_CHUNKS = B_SHARD // CHUNK  # 4
DBLK = D // 128              # 32
TSUB = CHUNK // 128          # 4
HK = 4                       # h tiles split in 4 along depth (d)
DHALF = DBLK // HK           # 8 d-blocks per quarter
PIPE = 2                     # transpose->matmul software pipeline offset

F32 = mybir.dt.float32
F32R = mybir.dt.float32r
F16 = mybir.dt.float16
BF16 = mybir.dt.bfloat16
U32 = mybir.dt.uint32
I32 = mybir.dt.int32
AF = mybir.ActivationFunctionType


def _build():
    nc = bacc.Bacc("TRN2", target_bir_lowering=False, debug=False,
                   num_devices=N_CORES)
    h_d = nc.dram_tensor("h", [B_SHARD, D], F32, kind="ExternalInput")
    w_d = nc.dram_tensor("weight", [E, D], F32, kind="ExternalInput")
    b_d = nc.dram_tensor("bias", [E], F32, kind="ExternalInput")
    ow_d = nc.dram_tensor("topk_w", [B_SHARD, 2], F32, kind="ExternalOutput")
    oi_d = nc.dram_tensor("topk_idx", [B_SHARD, 2], I32, kind="ExternalOutput")

    with TileContext(nc) as tc:
        with (
            tc.tile_pool(name="const", bufs=1) as constp,
            tc.tile_pool(name="hnat", bufs=2) as hnatp,
            tc.tile_pool(name="ht0", bufs=4) as ht0p,
            tc.tile_pool(name="ht1", bufs=4) as ht1p,
            tc.tile_pool(name="small", bufs=2) as smallp,
            tc.tile_pool(name="outp", bufs=2) as outp,
        ):
            hn_live = {}

            def emit_h_dmas(c):
                for k in range(HK):
                    for s in range(TSUB):
                        t = hnatp.tile([128, D // HK], F32,
                                       name=f"hn_{c}_{s}_{k}",
                                       tag=f"hn_{s}_{k}")
                        t0 = c * CHUNK + s * 128
                        d0 = k * (D // HK)
                        nc.sync.dma_start(out=t[:],
                                          in_=h_d[t0:t0 + 128,
                                                  d0:d0 + D // HK])
                        hn_live[(c, s, k)] = t

            bias_sb = constp.tile([E, 1], F32, name="bias_sb")
            nc.sync.dma_start(out=bias_sb[:],
                              in_=b_d.ap().rearrange("(e o) -> e o", o=1))
            wnat = constp.tile([E, D], F32, name="wnat")
            for q in range(4):
                nc.sync.dma_start(out=wnat[:, 1024 * q:1024 * (q + 1)],
                                  in_=w_d[:, 1024 * q:1024 * (q + 1)])
            emit_h_dmas(0)

            ident = constp.tile([128, 128], F32, name="ident")
            make_identity(nc, ident[:])

            # --- W setup: all 32 W^T blocks into 4 borrowed PSUM tiles,
            # then 4 wide copies per output (f32r hi, f32r lo, bf16) ---
            wcat = constp.tile([128, DBLK * 128], F16, name="wcat")
            wb = constp.tile([128, DBLK * E], BF16, name="wb")
            wtmp = constp.tile([128, CHUNK], F32, name="wtmp")
            with tc.tile_pool(name="wps", bufs=4, space="PSUM") as wpsp:
                wps = [wpsp.tile([128, CHUNK], F32, name=f"wps_{j}", tag="wps")
                       for j in range(4)]
                wcv = wcat[:].rearrange("p (d t) -> p d t", t=128)
                for d in range(DBLK):
                    j, m = d // 8, d % 8
                    nc.tensor.transpose(wps[j][:, E * m:E * (m + 1)],
                                        wnat[:, 128 * d:128 * (d + 1)],
                                        ident[0:E, 0:E])
                    if m == 7:
                        c0 = wcv[:, 8 * j:8 * (j + 1), 0:E]
                        c1 = wcv[:, 8 * j:8 * (j + 1), E:128]
                        nc.scalar.copy(c0, wps[j][:])
                        nc.vector.tensor_sub(wtmp[:], wps[j][:], c0)
                        nc.vector.tensor_scalar(c1, wtmp[:], 2048.0,
                                                scalar2=None,
                                                op0=mybir.AluOpType.mult)
                        nc.scalar.copy(wb[:, CHUNK * j:CHUNK * (j + 1)],
                                       wps[j][:])

            with (
                tc.tile_pool(name="lpA", bufs=2, space="PSUM") as lpAp,
                tc.tile_pool(name="lpB", bufs=2, space="PSUM") as lpBp,
                tc.tile_pool(name="ltps", bufs=2, space="PSUM") as ltpsp,
                tc.tile_pool(name="tps", bufs=2, space="PSUM") as tpsp,
            ):
                lp_live = {}

                def emit_tail(c):
                    # logits^T = (w1.h0 + bias) + w0.h0 + bf16(w).h1
                    lpA, lpB = lp_live.pop(c)
                    t1 = smallp.tile([E, CHUNK], F32, name=f"t1_{c}",
                                     tag="t1")
                    t2 = smallp.tile([E, CHUNK], F32, name=f"t2_{c}",
                                     tag="tmp")
                    lsb = smallp.tile([E, CHUNK], F32, name=f"lsb_{c}",
                                      tag="tmp")
                    nc.scalar.activation(t1[:], lpA[0:E, :], AF.Identity,
                                         bias=bias_sb[:])
                    nc.vector.scalar_tensor_tensor(
                        t2[:], lpA[E:128, :], 1.0 / 2048.0, t1[:],
                        op0=mybir.AluOpType.mult, op1=mybir.AluOpType.add)
                    nc.vector.tensor_add(lsb[:], t2[:], lpB[:])

                    # back to [128 tok, 64 e]; top-2; renorm (chunk-wide)
                    m8 = smallp.tile([128, 8 * TSUB], F32, name=f"m8_{c}",
                                     tag="m8")
                    i8 = smallp.tile([128, 8 * TSUB], U32, name=f"i8_{c}",
                                     tag="i8")
                    lt = ltpsp.tile([128, TSUB * E], F32, name=f"lt_{c}",
                                    tag="lt")
                    for s in range(TSUB):
                        nc.tensor.transpose(lt[:, E * s:E * (s + 1)],
                                            lsb[:, 128 * s:128 * (s + 1)],
                                            ident[0:E, 0:E])
                    for s in range(TSUB):
                        nc.vector.max_with_indices(m8[:, 8 * s:8 * s + 8],
                                                   i8[:, 8 * s:8 * s + 8],
                                                   lt[:, E * s:E * (s + 1)])

                    oww = outp.tile([128, 2 * TSUB], F32, name=f"oww_{c}",
                                    tag="oww")
                    oii = outp.tile([128, 2 * TSUB], I32, name=f"oii_{c}",
                                    tag="oii")
                    dd = smallp.tile([128, TSUB], F32, name=f"dd_{c}",
                                     tag="dd")
                    m8v = m8[:].rearrange("p (s e) -> p s e", e=8)
                    i8v = i8[:].rearrange("p (s e) -> p s e", e=8)
                    owv = oww[:].rearrange("p (s c) -> p s c", c=2)
                    # w1 = sigmoid(l1 - l2) = 1/(1+exp(l2-l1)); w2 = 1 - w1
                    nc.vector.tensor_sub(dd[:], m8v[:, :, 0:1],
                                         m8v[:, :, 1:2])
                    nc.scalar.activation(owv[:, :, 0:1], dd[:], AF.Sigmoid)
                    nc.vector.tensor_scalar(owv[:, :, 1:2], owv[:, :, 0:1],
                                            -1.0, 1.0,
                                            op0=mybir.AluOpType.mult,
                                            op1=mybir.AluOpType.add)
                    nc.vector.tensor_copy(oii[:], i8v[:, :, 0:2].bitcast(I32))

                    t0 = c * CHUNK
                    nc.sync.dma_start(
                        out=ow_d[t0:t0 + CHUNK, :].rearrange(
                            "(s p) c -> p s c", p=128),
                        in_=oww[:])
                    nc.scalar.dma_start(
                        out=oi_d[t0:t0 + CHUNK, :].rearrange(
                            "(s p) c -> p s c", p=128),
                        in_=oii[:])

                for c in range(N_CHUNKS):
                    if c + 1 < N_CHUNKS:
                        emit_h_dmas(c + 1)
                    hn = [[hn_live.pop((c, s, k)) for k in range(HK)]
                          for s in range(TSUB)]

                    lpA = lpAp.tile([128, CHUNK], F32, name=f"lpA_{c}",
                                    tag="lpA")
                    lpB = lpBp.tile([E, CHUNK], F32, name=f"lpB_{c}",
                                    tag="lpB")
                    lp_live[c] = (lpA, lpB)
                    ht_live = {}
                    for step in range(DBLK + PIPE):
                        if step < DBLK:
                            d = step
                            k, dk = d // DHALF, d % DHALF
                            tp = tpsp.tile([128, CHUNK], F32,
                                           name=f"tp_{c}_{d}", tag="tp")
                            for s in range(TSUB):
                                nc.tensor.transpose(
                                    tp[:, 128 * s:128 * (s + 1)],
                                    hn[s][k][:, 128 * dk:128 * (dk + 1)],
                                    ident[:])
                            ht0 = ht0p.tile([128, CHUNK], F16,
                                            name=f"ht0_{c}_{d}", tag="ht0")
                            ht1 = ht1p.tile([128, CHUNK], BF16,
                                            name=f"ht1_{c}_{d}", tag="ht1")
                            nc.scalar.copy(ht0[:], tp[:])
                            nc.vector.tensor_sub(ht1[:], tp[:], ht0[:])
                            ht_live[d] = (ht0, ht1)
                        if step >= PIPE:
                            d = step - PIPE
                            ht0, ht1 = ht_live.pop(d)
                            nc.tensor.matmul(lpA[:],
                                             wcat[:, 128 * d:128 * (d + 1)],
                                             ht0[:], start=(d == 0),
                                             stop=(d == DBLK - 1))
                            nc.tensor.matmul(lpB[:],
                                             wb[:, E * d:E * (d + 1)],
                                             ht1[:], start=(d == 0),
                                             stop=(d == DBLK - 1))
                        if step == 3 and c > 0:
                            emit_tail(c - 1)
                emit_tail(N_CHUNKS - 1)

    nc.compile()
    return nc


_NC = None


def _get_nc():
    global _NC
    if _NC is None:
        _NC = _build()
    return _NC


def run(h, weight, bias, trace=False):
    nc = _get_nc()
    h = np.ascontiguousarray(h, dtype=np.float32)
    weight = np.ascontiguousarray(weight, dtype=np.float32)
    bias = np.ascontiguousarray(bias, dtype=np.float32)
    in_maps = [{"h": h[i * B_SHARD:(i + 1) * B_SHARD], "weight": weight,
                "bias": bias} for i in range(N_CORES)]
    res = run_bass_kernel_spmd(nc, in_maps, list(range(N_CORES)), trace=trace)
    tw = np.concatenate([res.results[i]["topk_w"] for i in range(N_CORES)], 0)
    ti = np.concatenate([res.results[i]["topk_idx"] for i in range(N_CORES)], 0)
    return (tw.astype(np.float32), ti.astype(np.int32)), res


def kernel(h, weight, bias):
    (tw, ti), _ = run(h, weight, bias)
    return tw, ti


# revision 38
# speedup vs baseline: 1.0167x; 1.0084x over previous
"""MoE gate kernel for TRN2: logits = h @ W.T + bias; softmax; top-2; renorm.

Data-parallel over 8 NeuronCores: token dim B=16384 sharded to 2048/core,
weight (64, 4096) + bias replicated.

Near-exact fp32 matmul via fp16 splitting: h = h0 + h1 with h0 = fp16(h)
and h1 = bf16(h - h0); W = w0 + w1 with w0 = fp16(W) and w1 scaled by
2^11 into fp16 range (w1s = fp16((W - w0) * 2048)).
  pass A (fp16): [w0 | w1s]^T @ h0 -> w0.h0 (rows 0:64) and
                 2048 * w1.h0 (rows 64:128, descaled in the combine)
  pass B (bf16): bf16(W)^T @ bf16(h1)
All products are exact in the PE (fp32 PSUM accumulation); only the bf16
storage of h1 rounds, reproducing fp32 logits to ~1e-6 so the top-2
indices match the fp32 reference, while both passes stream 2-byte data
at the full 1 cycle/row PE rate (4x the fp32 matmul rate).

Pipeline per core: h loaded naturally in half-depth tiles [128, 2048]
(frees buffers mid-chunk for deep DMA prefetch; next chunk's loads are
emitted before this chunk's output stores so the sync queue never
stalls). PE fp32-transposes build hT blocks; scalar rounds PSUM->SBUF
to f32r (h0), vector computes the bf16 residual (h1); matmuls run PIPE
d-blocks behind the transposes. W setup batches all 32 transposes into
4 borrowed PSUM tiles then splits with 4 wide copies, overlapped with
the first h DMAs. Top-2 via vector max8/idx8; renorm w1=1/(1+e),
w2=e/(1+e) with e=exp(l2-l1) -- softmax-renorm restricted to the top 2.
"""
import numpy as np
import concourse.bacc as bacc
import concourse.mybir as mybir
from concourse.tile import TileContext
from concourse.bass_utils import run_bass_kernel_spmd
from concourse.masks import make_identity

N_CORES = 8
B = 16384
D = 4096
E = 64
B_SHARD = B // N_CORES      # 2048
CHUNK = 512
N_CHUNKS = B_SHARD // CHUNK  # 4
DBLK = D // 128              # 32
TSUB = CHUNK // 128          # 4
HK = 8                       # h tiles split in 8 along depth (d)
DHALF = DBLK // HK           # 4 d-blocks per slice
PIPE = 2                     # transpose->matmul software pipeline offset

F32 = mybir.dt.float32
F32R = mybir.dt.float32r
F16 = mybir.dt.float16
BF16 = mybir.dt.bfloat16
U32 = mybir.dt.uint32
I32 = mybir.dt.int32
AF = mybir.ActivationFunctionType


def _build():
    nc = bacc.Bacc("TRN2", target_bir_lowering=False, debug=False,
                   num_devices=N_CORES)
    h_d = nc.dram_tensor("h", [B_SHARD, D], F32, kind="ExternalInput")
    w_d = nc.dram_tensor("weight", [E, D], F32, kind="ExternalInput")
    b_d = nc.dram_tensor("bias", [E], F32, kind="ExternalInput")
    ow_d = nc.dram_tensor("topk_w", [B_SHARD, 2], F32, kind="ExternalOutput")
    oi_d = nc.dram_tensor("topk_idx", [B_SHARD, 2], I32, kind="ExternalOutput")

    with TileContext(nc) as tc:
        with (
            tc.tile_pool(name="const", bufs=1) as constp,
            tc.tile_pool(name="hnat", bufs=2) as hnatp,
            tc.tile_pool(name="ht0", bufs=4) as ht0p,
            tc.tile_pool(name="ht1", bufs=4) as ht1p,
            tc.tile_pool(name="small", bufs=2) as smallp,
            tc.tile_pool(name="outp", bufs=2) as outp,
        ):
            hn_live = {}

            def emit_h_dmas(c):
                for k in range(HK):
                    for s in range(TSUB):
                        t = hnatp.tile([128, D // HK], F32,
                                       name=f"hn_{c}_{s}_{k}",
                                       tag=f"hn_{s}_{k}")
                        t0 = c * CHUNK + s * 128
                        d0 = k * (D // HK)
                        nc.sync.dma_start(out=t[:],
                                          in_=h_d[t0:t0 + 128,
                                                  d0:d0 + D // HK])
                        hn_live[(c, s, k)] = t

            bias_sb = constp.tile([E, 1], F32, name="bias_sb")
            nc.sync.dma_start(out=bias_sb[:],
                              in_=b_d.ap().rearrange("(e o) -> e o", o=1))
            wnat = constp.tile([E, D], F32, name="wnat")
            for q in range(4):
                nc.sync.dma_start(out=wnat[:, 1024 * q:1024 * (q + 1)],
                                  in_=w_d[:, 1024 * q:1024 * (q + 1)])
            emit_h_dmas(0)

            ident = constp.tile([128, 128], F32, name="ident")
            make_identity(nc, ident[:])

            # --- W setup: all 32 W^T blocks into 4 borrowed PSUM tiles,
            # then 4 wide copies per output (f32r hi, f32r lo, bf16) ---
            wcat = constp.tile([128, DBLK * 128], F16, name="wcat")
            wb = constp.tile([128, DBLK * E], BF16, name="wb")
            wtmp = constp.tile([128, CHUNK], F32, name="wtmp")
            with tc.tile_pool(name="wps", bufs=4, space="PSUM") as wpsp:
                wps = [wpsp.tile([128, CHUNK], F32, name=f"wps_{j}", tag="wps")
                       for j in range(4)]
                wcv = wcat[:].rearrange("p (d t) -> p d t", t=128)
                for d in range(DBLK):
                    j, m = d // 8, d % 8
                    nc.tensor.transpose(wps[j][:, E * m:E * (m + 1)],
                                        wnat[:, 128 * d:128 * (d + 1)],
                                        ident[0:E, 0:E])
                    if m == 7:
                        c0 = wcv[:, 8 * j:8 * (j + 1), 0:E]
                        c1 = wcv[:, 8 * j:8 * (j + 1), E:128]
                        nc.scalar.copy(c0, wps[j][:])
                        nc.vector.tensor_sub(wtmp[:], wps[j][:], c0)
                        nc.vector.tensor_scalar(c1, wtmp[:], 2048.0,
                                                scalar2=None,
                                                op0=mybir.AluOpType.mult)
                        nc.scalar.copy(wb[:, CHUNK * j:CHUNK * (j + 1)],
                                       wps[j][:])

            with (
                tc.tile_pool(name="lpA", bufs=2, space="PSUM") as lpAp,
                tc.tile_pool(name="lpB", bufs=2, space="PSUM") as lpBp,
                tc.tile_pool(name="ltps", bufs=2, space="PSUM") as ltpsp,
                tc.tile_pool(name="tps", bufs=2, space="PSUM") as tpsp,
            ):
                lp_live = {}

                def emit_tail(c):
                    # logits^T = (w1.h0 + bias) + w0.h0 + bf16(w).h1
                    lpA, lpB = lp_live.pop(c)
                    t1 = smallp.tile([E, CHUNK], F32, name=f"t1_{c}",
                                     tag="t1")
                    t2 = smallp.tile([E, CHUNK], F32, name=f"t2_{c}",
                                     tag="tmp")
                    lsb = smallp.tile([E, CHUNK], F32, name=f"lsb_{c}",
                                      tag="tmp")
                    nc.scalar.activation(t1[:], lpA[0:E, :], AF.Identity,
                                         bias=bias_sb[:])
                    nc.vector.scalar_tensor_tensor(
                        t2[:], lpA[E:128, :], 1.0 / 2048.0, t1[:],
                        op0=mybir.AluOpType.mult, op1=mybir.AluOpType.add)
                    nc.vector.tensor_add(lsb[:], t2[:], lpB[:])

                    # back to [128 tok, 64 e]; top-2; renorm (chunk-wide)
                    m8 = smallp.tile([128, 8 * TSUB], F32, name=f"m8_{c}",
                                     tag="m8")
                    i8 = smallp.tile([128, 8 * TSUB], U32, name=f"i8_{c}",
                                     tag="i8")
                    lt = ltpsp.tile([128, TSUB * E], F32, name=f"lt_{c}",
                                    tag="lt")
                    for s in range(TSUB):
                        nc.tensor.transpose(lt[:, E * s:E * (s + 1)],
                                            lsb[:, 128 * s:128 * (s + 1)],
                                            ident[0:E, 0:E])
                    for s in range(TSUB):
                        nc.vector.max_with_indices(m8[:, 8 * s:8 * s + 8],
                                                   i8[:, 8 * s:8 * s + 8],
                                                   lt[:, E * s:E * (s + 1)])

                    oww = outp.tile([128, 2 * TSUB], F32, name=f"oww_{c}",
                                    tag="oww")
                    oii = outp.tile([128, 2 * TSUB], I32, name=f"oii_{c}",
                                    tag="oii")
                    dd = smallp.tile([128, TSUB], F32, name=f"dd_{c}",
                                     tag="dd")
                    m8v = m8[:].rearrange("p (s e) -> p s e", e=8)
                    i8v = i8[:].rearrange("p (s e) -> p s e", e=8)
                    owv = oww[:].rearrange("p (s c) -> p s c", c=2)
                    # w1 = sigmoid(l1 - l2) = 1/(1+exp(l2-l1)); w2 = 1 - w1
                    nc.vector.tensor_sub(dd[:], m8v[:, :, 0:1],
                                         m8v[:, :, 1:2])
                    nc.scalar.activation(owv[:, :, 0:1], dd[:], AF.Sigmoid)
                    nc.vector.tensor_scalar(owv[:, :, 1:2], owv[:, :, 0:1],
                                            -1.0, 1.0,
                                            op0=mybir.AluOpType.mult,
                                            op1=mybir.AluOpType.add)
                    nc.vector.tensor_copy(oii[:], i8v[:, :, 0:2].bitcast(I32))

                    t0 = c * CHUNK
                    nc.sync.dma_start(
                        out=ow_d[t0:t0 + CHUNK, :].rearrange(
                            "(s p) c -> p s c", p=128),
                        in_=oww[:])
                    nc.scalar.dma_start(
                        out=oi_d[t0:t0 + CHUNK, :].rearrange(
                            "(s p) c -> p s c", p=128),
                        in_=oii[:])

                for c in range(N_CHUNKS):
                    if c + 1 < N_CHUNKS:
                        emit_h_dmas(c + 1)
                    hn = [[hn_live.pop((c, s, k)) for k in range(HK)]
                          for s in range(TSUB)]

                    lpA = lpAp.tile([128, CHUNK], F32, name=f"lpA_{c}",
                                    tag="lpA")
                    lpB = lpBp.tile([E, CHUNK], F32, name=f"lpB_{c}",
                                    tag="lpB")
                    lp_live[c] = (lpA, lpB)
                    ht_live = {}
                    for step in range(DBLK + PIPE):
                        if step < DBLK:
                            d = step
                            k, dk = d // DHALF, d % DHALF
                            tp = tpsp.tile([128, CHUNK], F32,
                                           name=f"tp_{c}_{d}", tag="tp")
                            for s in range(TSUB):
                                nc.tensor.transpose(
                                    tp[:, 128 * s:128 * (s + 1)],
                                    hn[s][k][:, 128 * dk:128 * (dk + 1)],
                                    ident[:])
                            ht0 = ht0p.tile([128, CHUNK], F16,
                                            name=f"ht0_{c}_{d}", tag="ht0")
                            ht1 = ht1p.tile([128, CHUNK], BF16,
                                            name=f"ht1_{c}_{d}", tag="ht1")
                            nc.scalar.copy(ht0[:], tp[:])
                            nc.vector.tensor_sub(ht1[:], tp[:], ht0[:])
                            ht_live[d] = (ht0, ht1)
                        if step >= PIPE:
                            d = step - PIPE
                            ht0, ht1 = ht_live.pop(d)
                            nc.tensor.matmul(lpA[:],
                                             wcat[:, 128 * d:128 * (d + 1)],
                                             ht0[:], start=(d == 0),
                                             stop=(d == DBLK - 1))
                            nc.tensor.matmul(lpB[:],
                                             wb[:, E * d:E * (d + 1)],
                                             ht1[:], start=(d == 0),
                                             stop=(d == DBLK - 1))
                        if step == 3 and c > 0:
                            emit_tail(c - 1)
                emit_tail(N_CHUNKS - 1)

    nc.compile()
    return nc


_NC = None


def _get_nc():
    global _NC
    if _NC is None:
        _NC = _build()
    return _NC


def run(h, weight, bias, trace=False):
    nc = _get_nc()
    h = np.ascontiguousarray(h, dtype=np.float32)
    weight = np.ascontiguousarray(weight, dtype=np.float32)
    bias = np.ascontiguousarray(bias, dtype=np.float32)
    in_maps = [{"h": h[i * B_SHARD:(i + 1) * B_SHARD], "weight": weight,
                "bias": bias} for i in range(N_CORES)]
    res = run_bass_kernel_spmd(nc, in_maps, list(range(N_CORES)), trace=trace)
    tw = np.concatenate([res.results[i]["topk_w"] for i in range(N_CORES)], 0)
    ti = np.concatenate([res.results[i]["topk_idx"] for i in range(N_CORES)], 0)
    return (tw.astype(np.float32), ti.astype(np.int32)), res


def kernel(h, weight, bias):
    (tw, ti), _ = run(h, weight, bias)
    return tw, ti


# revision 39
# speedup vs baseline: 1.0197x; 1.0030x over previous
"""MoE gate kernel for TRN2: logits = h @ W.T + bias; softmax; top-2; renorm.

Data-parallel over 8 NeuronCores: token dim B=16384 sharded to 2048/core,
weight (64, 4096) + bias replicated.

Near-exact fp32 matmul via fp16 splitting: h = h0 + h1 with h0 = fp16(h)
and h1 = bf16(h - h0); W = w0 + w1 with w0 = fp16(W) and w1 scaled by
2^11 into fp16 range (w1s = fp16((W - w0) * 2048)).
  pass A (fp16): [w0 | w1s]^T @ h0 -> w0.h0 (rows 0:64) and
                 2048 * w1.h0 (rows 64:128, descaled in the combine)
  pass B (bf16): bf16(W)^T @ bf16(h1)
All products are exact in the PE (fp32 PSUM accumulation); only the bf16
storage of h1 rounds, reproducing fp32 logits to ~1e-6 so the top-2
indices match the fp32 reference, while both passes stream 2-byte data
at the full 1 cycle/row PE rate (4x the fp32 matmul rate).

Pipeline per core: h loaded naturally in eighth-depth tiles [128, 512]
(fast first arrival; buffers free mid-chunk for deep DMA prefetch; next
chunk's loads are emitted before this chunk's output stores so the sync
queue never stalls). PE fp32-transposes build hT blocks; scalar rounds
PSUM->SBUF to fp16 (h0), vector computes the bf16 residual (h1);
matmuls run PIPE d-blocks behind the transposes. W setup batches all 32
transposes into 4 borrowed PSUM tiles then splits with wide copies,
overlapped with the first h DMAs. Top-2 via vector max8/idx8; renorm
w1 = sigmoid(l1-l2), w2 = 1-w1 -- exactly softmax-renorm restricted to
the top 2 (full-softmax denominator cancels).
"""
import numpy as np
import concourse.bacc as bacc
import concourse.mybir as mybir
from concourse.tile import TileContext
from concourse.bass_utils import run_bass_kernel_spmd
from concourse.masks import make_identity

N_CORES = 8
B = 16384
D = 4096
E = 64
B_SHARD = B // N_CORES      # 2048
CHUNK = 512
N_CHUNKS = B_SHARD // CHUNK  # 4
DBLK = D // 128              # 32
TSUB = CHUNK // 128          # 4
HK = 8                       # h tiles split in 8 along depth (d)
DHALF = DBLK // HK           # 4 d-blocks per slice
PIPE = 2                     # transpose->matmul software pipeline offset

F32 = mybir.dt.float32
F32R = mybir.dt.float32r
F16 = mybir.dt.float16
BF16 = mybir.dt.bfloat16
U32 = mybir.dt.uint32
I32 = mybir.dt.int32
AF = mybir.ActivationFunctionType


def _build():
    nc = bacc.Bacc("TRN2", target_bir_lowering=False, debug=False,
                   num_devices=N_CORES)
    h_d = nc.dram_tensor("h", [B_SHARD, D], F32, kind="ExternalInput")
    w_d = nc.dram_tensor("weight", [E, D], F32, kind="ExternalInput")
    b_d = nc.dram_tensor("bias", [E], F32, kind="ExternalInput")
    ow_d = nc.dram_tensor("topk_w", [B_SHARD, 2], F32, kind="ExternalOutput")
    oi_d = nc.dram_tensor("topk_idx", [B_SHARD, 2], I32, kind="ExternalOutput")

    with TileContext(nc) as tc:
        with (
            tc.tile_pool(name="const", bufs=1) as constp,
            tc.tile_pool(name="hnat", bufs=2) as hnatp,
            tc.tile_pool(name="ht0", bufs=4) as ht0p,
            tc.tile_pool(name="ht1", bufs=4) as ht1p,
            tc.tile_pool(name="small", bufs=2) as smallp,
            tc.tile_pool(name="outp", bufs=2) as outp,
        ):
            hn_live = {}

            def emit_h_dmas(c):
                for k in range(HK):
                    for s in range(TSUB):
                        t = hnatp.tile([128, D // HK], F32,
                                       name=f"hn_{c}_{s}_{k}",
                                       tag=f"hn_{s}_{k}")
                        t0 = c * CHUNK + s * 128
                        d0 = k * (D // HK)
                        nc.sync.dma_start(out=t[:],
                                          in_=h_d[t0:t0 + 128,
                                                  d0:d0 + D // HK])
                        hn_live[(c, s, k)] = t

            bias_sb = constp.tile([E, 1], F32, name="bias_sb")
            nc.sync.dma_start(out=bias_sb[:],
                              in_=b_d.ap().rearrange("(e o) -> e o", o=1))
            wnat = constp.tile([E, D], F32, name="wnat")
            for q in range(4):
                nc.sync.dma_start(out=wnat[:, 1024 * q:1024 * (q + 1)],
                                  in_=w_d[:, 1024 * q:1024 * (q + 1)])
            emit_h_dmas(0)

            ident = constp.tile([128, 128], F32, name="ident")
            make_identity(nc, ident[:])

            # --- W setup: all 32 W^T blocks into 4 borrowed PSUM tiles,
            # then 4 wide copies per output (f32r hi, f32r lo, bf16) ---
            wcat = constp.tile([128, DBLK * 128], F16, name="wcat")
            wb = constp.tile([128, DBLK * E], BF16, name="wb")
            wtmp = constp.tile([128, CHUNK], F32, name="wtmp")
            with tc.tile_pool(name="wps", bufs=4, space="PSUM") as wpsp:
                wps = [wpsp.tile([128, CHUNK], F32, name=f"wps_{j}", tag="wps")
                       for j in range(4)]
                wcv = wcat[:].rearrange("p (d t) -> p d t", t=128)
                for d in range(DBLK):
                    j, m = d // 8, d % 8
                    nc.tensor.transpose(wps[j][:, E * m:E * (m + 1)],
                                        wnat[:, 128 * d:128 * (d + 1)],
                                        ident[0:E, 0:E])
                    if m == 7:
                        c0 = wcv[:, 8 * j:8 * (j + 1), 0:E]
                        c1 = wcv[:, 8 * j:8 * (j + 1), E:128]
                        nc.scalar.copy(c0, wps[j][:])
                        nc.vector.tensor_sub(wtmp[:], wps[j][:], c0)
                        nc.vector.tensor_scalar(c1, wtmp[:], 2048.0,
                                                scalar2=None,
                                                op0=mybir.AluOpType.mult)
                        nc.scalar.copy(wb[:, CHUNK * j:CHUNK * (j + 1)],
                                       wps[j][:])

            with (
                tc.tile_pool(name="lpA", bufs=2, space="PSUM") as lpAp,
                tc.tile_pool(name="lpB", bufs=2, space="PSUM") as lpBp,
                tc.tile_pool(name="ltps", bufs=2, space="PSUM") as ltpsp,
                tc.tile_pool(name="tps", bufs=2, space="PSUM") as tpsp,
            ):
                lp_live = {}

                def emit_tail(c):
                    # logits^T = (w1.h0 + bias) + w0.h0 + bf16(w).h1
                    lpA, lpB = lp_live.pop(c)
                    t1 = smallp.tile([E, CHUNK], F32, name=f"t1_{c}",
                                     tag="t1")
                    t2 = smallp.tile([E, CHUNK], F32, name=f"t2_{c}",
                                     tag="tmp")
                    lsb = smallp.tile([E, CHUNK], F32, name=f"lsb_{c}",
                                      tag="tmp")
                    nc.scalar.activation(t1[:], lpA[0:E, :], AF.Identity,
                                         bias=bias_sb[:])
                    nc.vector.scalar_tensor_tensor(
                        t2[:], lpA[E:128, :], 1.0 / 2048.0, t1[:],
                        op0=mybir.AluOpType.mult, op1=mybir.AluOpType.add)
                    nc.vector.tensor_add(lsb[:], t2[:], lpB[:])

                    # back to [128 tok, 64 e]; top-2; renorm (chunk-wide)
                    m8 = smallp.tile([128, 8 * TSUB], F32, name=f"m8_{c}",
                                     tag="m8")
                    i8 = smallp.tile([128, 8 * TSUB], U32, name=f"i8_{c}",
                                     tag="i8")
                    lt = ltpsp.tile([128, TSUB * E], F32, name=f"lt_{c}",
                                    tag="lt")
                    for s in range(TSUB):
                        nc.tensor.transpose(lt[:, E * s:E * (s + 1)],
                                            lsb[:, 128 * s:128 * (s + 1)],
                                            ident[0:E, 0:E])
                    for s in range(TSUB):
                        nc.vector.max_with_indices(m8[:, 8 * s:8 * s + 8],
                                                   i8[:, 8 * s:8 * s + 8],
                                                   lt[:, E * s:E * (s + 1)])

                    oww = outp.tile([128, 2 * TSUB], F32, name=f"oww_{c}",
                                    tag="oww")
                    oii = outp.tile([128, 2 * TSUB], I32, name=f"oii_{c}",
                                    tag="oii")
                    dd = smallp.tile([128, TSUB], F32, name=f"dd_{c}",
                                     tag="dd")
                    m8v = m8[:].rearrange("p (s e) -> p s e", e=8)
                    i8v = i8[:].rearrange("p (s e) -> p s e", e=8)
                    owv = oww[:].rearrange("p (s c) -> p s c", c=2)
                    # w1 = sigmoid(l1 - l2) = 1/(1+exp(l2-l1)); w2 = 1 - w1
                    nc.vector.tensor_sub(dd[:], m8v[:, :, 0:1],
                                         m8v[:, :, 1:2])
                    nc.scalar.activation(owv[:, :, 0:1], dd[:], AF.Sigmoid)
                    nc.vector.tensor_scalar(owv[:, :, 1:2], owv[:, :, 0:1],
                                            -1.0, 1.0,
                                            op0=mybir.AluOpType.mult,
                                            op1=mybir.AluOpType.add)
                    nc.vector.tensor_copy(oii[:], i8v[:, :, 0:2].bitcast(I32))

                    t0 = c * CHUNK
                    nc.sync.dma_start(
                        out=ow_d[t0:t0 + CHUNK, :].rearrange(
                            "(s p) c -> p s c", p=128),
                        in_=oww[:])
                    nc.scalar.dma_start(
                        out=oi_d[t0:t0 + CHUNK, :].rearrange(
                            "(s p) c -> p s c", p=128),
                        in_=oii[:])

                for c in range(N_CHUNKS):
                    if c + 1 < N_CHUNKS:
                        emit_h_dmas(c + 1)
                    hn = [[hn_live.pop((c, s, k)) for k in range(HK)]
                          for s in range(TSUB)]

                    lpA = lpAp.tile([128, CHUNK], F32, name=f"lpA_{c}",
                                    tag="lpA")
                    lpB = lpBp.tile([E, CHUNK], F32, name=f"lpB_{c}",
                                    tag="lpB")
                    lp_live[c] = (lpA, lpB)
                    ht_live = {}
                    for step in range(DBLK + PIPE):
                        if step < DBLK:
                            d = step
                            k, dk = d // DHALF, d % DHALF
                            tp = tpsp.tile([128, CHUNK], F32,
                                           name=f"tp_{c}_{d}", tag="tp")
                            for s in range(TSUB):
                                nc.tensor.transpose(
                                    tp[:, 128 * s:128 * (s + 1)],
                                    hn[s][k][:, 128 * dk:128 * (dk + 1)],
                                    ident[:])
                            ht0 = ht0p.tile([128, CHUNK], F16,
                                            name=f"ht0_{c}_{d}", tag="ht0")
                            ht1 = ht1p.tile([128, CHUNK], BF16,
                                            name=f"ht1_{c}_{d}", tag="ht1")
                            nc.scalar.copy(ht0[:], tp[:])
                            nc.vector.tensor_sub(ht1[:], tp[:], ht0[:])
                            ht_live[d] = (ht0, ht1)
                        if step >= PIPE:
                            d = step - PIPE
                            ht0, ht1 = ht_live.pop(d)
                            nc.tensor.matmul(lpA[:],
                                             wcat[:, 128 * d:128 * (d + 1)],
                                             ht0[:], start=(d == 0),
                                             stop=(d == DBLK - 1))
                            nc.tensor.matmul(lpB[:],
                                             wb[:, E * d:E * (d + 1)],
                                             ht1[:], start=(d == 0),
                                             stop=(d == DBLK - 1))
                        if step == 3 and c > 0:
                            emit_tail(c - 1)
                emit_tail(N_CHUNKS - 1)

    nc.compile()
    return nc


_NC = None


def _get_nc():
    global _NC
    if _NC is None:
        _NC = _build()
    return _NC


def run(h, weight, bias, trace=False):
    nc = _get_nc()
    h = np.ascontiguousarray(h, dtype=np.float32)
    weight = np.ascontiguousarray(weight, dtype=np.float32)
    bias = np.ascontiguousarray(bias, dtype=np.float32)
    in_maps = [{"h": h[i * B_SHARD:(i + 1) * B_SHARD], "weight": weight,
                "bias": bias} for i in range(N_CORES)]
    res = run_bass_kernel_spmd(nc, in_maps, list(range(N_CORES)), trace=trace)
    tw = np.concatenate([res.results[i]["topk_w"] for i in range(N_CORES)], 0)
    ti = np.concatenate([res.results[i]["topk_idx"] for i in range(N_CORES)], 0)
    return (tw.astype(np.float32), ti.astype(np.int32)), res


def kernel(h, weight, bias):
    (tw, ti), _ = run(h, weight, bias)
    return tw, ti
